# revision 8
# baseline (speedup 1.0000x reference)
"""Trainium2 Bass kernel: Conv3x3(64->128) + channel LayerNorm + LIF spiking over T=4.

Full inputs: x [4, 8, 64, 64, 64], conv_w [128, 64, 3, 3], conv_b [128],
ln_w [128], ln_b [128].  Output: spikes [4, 8, 128, 64, 64] (0.0/1.0, f32).

Sharding: data-parallel over B=8 -> one batch element per NeuronCore, no
cross-core communication.

Per-core pipeline (per time step t):
  1. Conv as 6 PSUM-accumulated f32r matmuls per 512-pixel chunk (f32r
     streams 1 row/cycle vs 4 for fp32).  SBUF holds one padded image copy
     "AB" (rows 0-63 = padded x, rows 64-127 = shifted by one image row),
     so a K=128 matmul covers taps (0,k)+(1,k); the row-2 taps are three
     K=64 matmuls on the top half at col offsets 0/1/2.
  2. ScalarE drains PSUM pairs to SBUF with the conv bias.
  3. PE transpose (fp32, exact) per 128x128 block -> [pixel, channel].
  4. LayerNorm stats via bn_stats (256-col call = per-128-tile groups),
     rstd via Sqrt+reciprocal; apply as per-tile tensor_scalar
     (split DVE/Pool), producing z/2 in SBUF.
  5. LIF on Pool: v = vh + z/2 (vh stored pre-halved+masked);
     mh = (v<1)*0.5 in bf16; vh' = v*mh.
  6. PE transpose of mh (bf16, 1 cyc/row, exact) back to [channel, pixel];
     drain computes spike = 1 - 2*mh (split ScalarE/DVE); DMA out.
"""

import numpy as np

import concourse.bacc as bacc
import concourse.mybir as mybir
import concourse.tile as tile
from concourse import bass_utils

T, B, CIN, H, W = 4, 8, 64, 64, 64
COUT = 128
HP, WP = H + 2, W + 2  # padded image: 66 x 66
NF = HP * WP  # 4356
NPIX = H * W  # 4096
NCHUNK = 8
CROWS = H // NCHUNK  # 8 image rows per chunk
CH_PIX = NPIX // NCHUNK  # 512
NTILE = NPIX // 128  # 32 transposed tiles per image
LN_EPS = 1e-5

F32 = mybir.dt.float32
F32R = mybir.dt.float32r
BF16 = mybir.dt.bfloat16

# engine-assignment knobs (tiles out of 32 / pairs out of 4)
APPLY_DVE = 20  # LN-apply tiles on DVE; rest on Pool
BACKT_SC_PAIRS = 2  # back-transpose drain pairs on ScalarE; rest on DVE
PTCOPY_SC = 8  # pt->yt copies (of 8 chunks) on ScalarE; rest on DVE
BN256 = True  # bn_stats per 256 cols (groups = 128-halves) vs per 128 + combine

LAST_EXEC_TIME_NS = None

_module_cache = {}


def _build_module(trivial_ln: bool):
    nc = bacc.Bacc("TRN2", debug=False)
    op = mybir.AluOpType
    act = mybir.ActivationFunctionType

    xab = nc.dram_tensor("xab", [T, 128, NF], F32R, kind="ExternalInput")
    w012 = nc.dram_tensor("w012", [128, 3, COUT], F32R, kind="ExternalInput")
    w2x = nc.dram_tensor("w2x", [CIN, 3, COUT], F32R, kind="ExternalInput")
    cb = nc.dram_tensor("cb", [COUT, 1], F32, kind="ExternalInput")
    ident = nc.dram_tensor("ident", [128, 128], F32, kind="ExternalInput")
    if not trivial_ln:
        lnw_rep = nc.dram_tensor("lnw_rep", [128, NTILE, 128], F32, kind="ExternalInput")
        lnb_rep = nc.dram_tensor("lnb_rep", [128, NTILE, 128], F32, kind="ExternalInput")
    out = nc.dram_tensor("out", [T, COUT, NPIX], F32, kind="ExternalOutput")

    with tile.TileContext(nc) as tc:
        with (
            tc.tile_pool(name="const", bufs=1) as cpool,
            tc.tile_pool(name="xin", bufs=2) as xpool,
            tc.tile_pool(name="ysb", bufs=2) as ypool,
            tc.tile_pool(name="img", bufs=2) as ipool,
            tc.tile_pool(name="vbuf", bufs=1) as vpool,
            tc.tile_pool(name="mh", bufs=2) as mpool,
            tc.tile_pool(name="stats", bufs=2) as spool,
            tc.tile_pool(name="obuf", bufs=2) as opool,
            tc.tile_pool(name="psy", bufs=2, space="PSUM") as psy,
            tc.tile_pool(name="pst", bufs=2, space="PSUM") as pst,
            tc.tile_pool(name="pss", bufs=2, space="PSUM") as pss,
        ):
            w012_t = cpool.tile([128, 3, COUT], F32R)
            nc.sync.dma_start(w012_t[:, :, :], w012[:, :, :])
            w2x_t = cpool.tile([CIN, 3, COUT], F32R)
            nc.sync.dma_start(w2x_t[:, :, :], w2x[:, :, :])
            cb_t = cpool.tile([COUT, 1], F32)
            nc.sync.dma_start(cb_t[:, :], cb[:, :])
            id_t = cpool.tile([128, 128], F32)
            nc.sync.dma_start(id_t[:, :], ident[:, :])
            idb_t = cpool.tile([128, 128], BF16)
            nc.vector.tensor_copy(idb_t[:, :], id_t[:, :])
            eps_t = cpool.tile([128, 1], F32)
            nc.gpsimd.memset(eps_t[:, :], 4.0 * LN_EPS)
            one_t = cpool.tile([128, 1], F32)
            nc.gpsimd.memset(one_t[:, :], 1.0)
            if not trivial_ln:
                lnw_t = cpool.tile([128, NTILE, 128], F32)
                nc.sync.dma_start(lnw_t[:, :, :], lnw_rep[:, :, :])
                lnb_t = cpool.tile([128, NTILE, 128], F32)
                nc.sync.dma_start(lnb_t[:, :, :], lnb_rep[:, :, :])

            # persistent LIF state: v_prev * (v_prev<1) * 0.5, [pixel, C]
            vh = vpool.tile([128, NPIX], F32)

            for t in range(T):
                if t == 0:
                    nc.gpsimd.memset(vh[:, :], 0.0)
                xt = xpool.tile([128, NF], F32R, tag="xab")
                nc.sync.dma_start(xt[:, :], xab[t])
                xg = xt[:, :].rearrange("p (h w) -> p h w", w=WP)

                yt = ipool.tile([128, NPIX], F32, tag="yt")
                if BN256:
                    bn = spool.tile([128, 16, 6], F32, tag="bn")
                else:
                    bn = spool.tile([128, NTILE, 6], F32, tag="bn")

                for pair in range(4):
                    py = psy.tile([128, 1024], F32, tag="py")
                    for ci in range(2):
                        c = 2 * pair + ci
                        h0 = c * CROWS
                        pyc = py[:, ci * 512 : ci * 512 + 512]
                        for k in range(3):
                            nc.tensor.matmul(
                                pyc,
                                w012_t[:, k, :],
                                xg[:, h0 : h0 + CROWS, k : k + W],
                                start=(k == 0),
                                stop=False,
                            )
                        for d in range(3):
                            nc.tensor.matmul(
                                pyc,
                                w2x_t[:, d, :],
                                xg[0:CIN, h0 + 2 : h0 + 2 + CROWS, d : d + W],
                                start=False,
                                stop=(d == 2),
                            )
                    # drain pair + conv bias (per-partition = per-channel)
                    ysb = ypool.tile([128, 1024], F32, tag="ysb")
                    nc.scalar.activation(
                        ysb[:, :], py[:, :], act.Identity, bias=cb_t[:, 0:1]
                    )
                    for ci in range(2):
                        c = 2 * pair + ci
                        pt = pst.tile([128, 512], F32, tag="pt")
                        for j in range(4):
                            nc.tensor.transpose(
                                pt[:, j * 128 : (j + 1) * 128],
                                ysb[:, ci * 512 + j * 128 : ci * 512 + (j + 1) * 128],
                                id_t[:, :],
                            )
                        # stats from PSUM.  bn_stats groups = even/odd stream
                        # positions; the [p, a, b] view (a step 1, b step 128)
                        # interleaves two 128-col tiles so the groups are
                        # exactly the tiles.
                        if BN256:
                            for jj in range(2):
                                ptv = pt[:, jj * 256 : (jj + 1) * 256].rearrange(
                                    "p (b a) -> p a b", b=2
                                )
                                # raw InstBNStats: the bass wrapper's 3D
                                # batching contradicts the HW (always 6 outs)
                                nc.vector.add_instruction(
                                    mybir.InstBNStats(
                                        name=nc.vector.bass.get_next_instruction_name(),
                                        ins=[nc.vector.lower_ap(ptv)],
                                        outs=[
                                            nc.vector.lower_ap(bn[:, c * 2 + jj, :])
                                        ],
                                    )
                                )
                        else:
                            for j in range(4):
                                nc.vector.bn_stats(
                                    bn[:, c * 4 + j, :],
                                    pt[:, j * 128 : (j + 1) * 128],
                                )
                        # copy transposed y to SBUF
                        ytc = yt[:, c * CH_PIX : (c + 1) * CH_PIX]
                        if c < PTCOPY_SC:
                            nc.scalar.activation(ytc, pt[:, :], act.Copy)
                        else:
                            nc.vector.tensor_copy(ytc, pt[:, :])

                # LayerNorm scalars, per 128-pixel tile
                if BN256:
                    # groups = the two interleaved tiles: mean/M2 per tile
                    mu = bn[:, :, 1:6:3].rearrange("p a b -> p (a b)")  # [128,32]
                    m2 = bn[:, :, 2:6:3].rearrange("p a b -> p (a b)")
                    q = spool.tile([128, NTILE], F32, tag="q")
                    # q = 4*var = M2 * (4/128)
                    nc.vector.tensor_scalar(
                        q[:, :], m2, 4.0 / 128.0, None, op0=op.mult
                    )
                else:
                    # combine even/odd 64-groups per tile
                    mu_t = spool.tile([128, NTILE], F32, tag="mu")
                    nc.vector.tensor_tensor(
                        mu_t[:, :], bn[:, :, 1], bn[:, :, 4], op=op.add
                    )
                    nc.vector.tensor_scalar(
                        mu_t[:, :], mu_t[:, :], 0.5, None, op0=op.mult
                    )
                    dm = spool.tile([128, NTILE], F32, tag="dm")
                    nc.vector.tensor_tensor(
                        dm[:, :], bn[:, :, 1], bn[:, :, 4], op=op.subtract
                    )
                    d2 = spool.tile([128, NTILE], F32, tag="d2")
                    nc.vector.tensor_tensor(d2[:, :], dm[:, :], dm[:, :], op=op.mult)
                    m2s = spool.tile([128, NTILE], F32, tag="m2s")
                    nc.vector.tensor_tensor(
                        m2s[:, :], bn[:, :, 2], bn[:, :, 5], op=op.add
                    )
                    q = spool.tile([128, NTILE], F32, tag="q")
                    # 4*var = M2s/32 + dm^2
                    nc.vector.scalar_tensor_tensor(
                        q[:, :], m2s[:, :], 1.0 / 32.0, d2[:, :],
                        op0=op.mult, op1=op.add,
                    )
                    mu = mu_t[:, :]
                # rh2 = 0.5/sqrt(var+eps) = 1/sqrt(4var + 4eps)
                sd = spool.tile([128, NTILE], F32, tag="sd")
                nc.scalar.activation(sd[:, :], q[:, :], act.Sqrt, bias=eps_t[:, 0:1])
                rh2 = spool.tile([128, NTILE], F32, tag="rh2")
                nc.vector.reciprocal(rh2[:, :], sd[:, :])

                # apply LN -> z/2, in place on yt (per-tile scalars)
                for j in range(NTILE):
                    zj = yt[:, j * 128 : (j + 1) * 128]
                    eng = nc.vector if j < APPLY_DVE else nc.gpsimd
                    eng.tensor_scalar(
                        zj, zj, mu[:, j : j + 1], rh2[:, j : j + 1],
                        op0=op.subtract, op1=op.mult,
                    )
                if not trivial_ln:
                    ytv = yt[:, :].rearrange("p (a b) -> p a b", b=128)
                    nc.vector.tensor_tensor(ytv, ytv, lnw_t[:, :, :], op=op.mult)
                    nc.vector.tensor_tensor(ytv, ytv, lnb_t[:, :, :], op=op.add)

                # LIF (Pool): v = vh + z/2 ; mh = (v<1)*0.5 (bf16) ; vh' = v*mh
                vt = ipool.tile([128, NPIX], F32, tag="vt")
                nc.gpsimd.tensor_tensor(vt[:, :], vh[:, :], yt[:, :], op=op.add)
                mh = mpool.tile([128, NPIX], BF16, tag="mh")
                nc.gpsimd.tensor_scalar(
                    mh[:, :], vt[:, :], 1.0, 0.5, op0=op.is_lt, op1=op.mult
                )
                nc.gpsimd.tensor_tensor(vh[:, :], vt[:, :], mh[:, :], op=op.mult)

                # transpose mh back to [channel, pixel]; spike = 1 - 2*mh
                ob = opool.tile([128, NPIX], F32, tag="ob")
                for pair in range(4):
                    ps = pss.tile([128, 1024], BF16, tag="ps")
                    for jj in range(8):
                        nc.tensor.transpose(
                            ps[:, jj * 128 : (jj + 1) * 128],
                            mh[:, pair * 1024 + jj * 128 : pair * 1024 + (jj + 1) * 128],
                            idb_t[:, :],
                        )
                    obp = ob[:, pair * 1024 : (pair + 1) * 1024]
                    if pair < BACKT_SC_PAIRS:
                        nc.scalar.activation(
                            obp, ps[:, :], act.Identity, bias=one_t[:, 0:1], scale=-2.0
                        )
                    else:
                        nc.vector.tensor_scalar(
                            obp, ps[:, :], -2.0, 1.0, op0=op.mult, op1=op.add
                        )
                nc.sync.dma_start(out[t], ob[:, :])

    nc.finalize()
    return nc


def kernel(x, conv_w, conv_b, ln_w, ln_b):
    global LAST_EXEC_TIME_NS
    import os

    x = np.asarray(x, dtype=np.float32)
    conv_w = np.asarray(conv_w, dtype=np.float32)
    conv_b = np.asarray(conv_b, dtype=np.float32)
    ln_w = np.asarray(ln_w, dtype=np.float32)
    ln_b = np.asarray(ln_b, dtype=np.float32)

    trivial_ln = bool(np.all(ln_w == 1.0) and np.all(ln_b == 0.0))

    key = ("mod", trivial_ln)
    if key not in _module_cache:
        _module_cache[key] = _build_module(trivial_ln)
    nc = _module_cache[key]

    # host-side weight packing
    wt = conv_w.transpose(1, 0, 2, 3)  # [CIN, COUT, 3, 3]
    w012_h = np.zeros((128, 3, COUT), dtype=np.float32)
    for k in range(3):
        w012_h[:CIN, k, :] = wt[:, :, 0, k]
        w012_h[CIN:, k, :] = wt[:, :, 1, k]
    w2x_h = np.ascontiguousarray(wt[:, :, 2, :].transpose(0, 2, 1))  # [CIN,3,COUT]
    cb_h = conv_b.reshape(COUT, 1).astype(np.float32)
    ident_h = np.eye(128, dtype=np.float32)

    extra = {}
    if not trivial_ln:
        extra["lnw_rep"] = np.ascontiguousarray(
            np.broadcast_to(ln_w, (128, NTILE, 128)).astype(np.float32)
        )
        extra["lnb_rep"] = np.ascontiguousarray(
            np.broadcast_to(ln_b * 0.5, (128, NTILE, 128)).astype(np.float32)
        )

    # input packing: AB = [xpad ; xpad shifted one image row]
    in_maps = []
    for b in range(B):
        xpad = np.zeros((T, CIN, HP, WP), dtype=np.float32)
        xpad[:, :, 1 : 1 + H, 1 : 1 + W] = x[:, b]
        xpad = xpad.reshape(T, CIN, NF)
        ab = np.zeros((T, 128, NF), dtype=np.float32)
        ab[:, :CIN, :] = xpad
        ab[:, CIN:, : NF - WP] = xpad[:, :, WP:]
        m = {"xab": ab, "w012": w012_h, "w2x": w2x_h, "cb": cb_h, "ident": ident_h}
        m.update(extra)
        in_maps.append(m)

    trace = bool(os.environ.get("KERNEL_TRACE"))
    res = bass_utils.run_bass_kernel_spmd(
        nc, in_maps, core_ids=list(range(B)), trace=trace
    )
    LAST_EXEC_TIME_NS = res.exec_time_ns

    outs = [res.results[b]["out"].reshape(T, COUT, H, W) for b in range(B)]
    return np.stack(outs, axis=1)
